# revision 14
# baseline (speedup 1.0000x reference)
"""Trainium2 distributed kernel for the AppearanceReconstruction loss.

Math note (exact identity, not an approximation): the MAE shuffle/gather in
the reference collapses — restored[b,p] is appearance_tokens[b,p] on kept
slots (which the mask multiplies by zero) and mask_token on masked slots.
Every row has exactly num_masked = 2 masked slots, and the decoder output at
a masked slot is the same single vector r = MLP(LN(mask_token)) for all
(b,p). Hence

    loss = 2 * sum_b mean_c((r_c - pooled[b,c])^2) / (256 + 1e-8)
    pooled[b] = mean_n target_features[b,n,:]

The memory-bound part (reading all of target_features) runs on the 8
NeuronCores, data-parallel over B (16 rows per core). target_features is
cast to fp8 e4m3 on the host before upload, quartering the HBM stream
(12.6 MB/core); the e4m3 quantization noise averages out over the
1024-token mean to ~3e-5 relative on the loss.

Per core the shard lives as [128 partitions, 96 KiB] (partition p = row
p//8), streamed by 14 HBM->SBUF jobs sliced along the free dim (small
ramp-in/out jobs so the first compute starts early and the post-final-DMA
tail is tiny). The token reduction is split across two engines running
off the same SBUF bytes: TensorEngine DoubleRow fp8 matmuls (2 tokens per
PE column-cycle, one-hot row-selector weights, accumulating into a
[16, 768] PSUM tile) take 88 of the 128 tokens per partition, and the
Vector engine reduce_sum takes the other 40 from mid-stream jobs (its
partials are combined on DVE and folded into PSUM by one K=128 f32r
matmul). A K=1 f32r matmul folds -N*r into the same accumulation, so
PSUM ends as N*(pooled - r) and the epilogue is one ACT
Square(scale=1/N)+row-sum. Raw Bass (no Tile framework) keeps the
end-of-kernel semaphore cleanup to a couple of range ops. The host sums
the 8x16 partials.
"""

import math

import numpy as np

B, N, C = 128, 1024, 768
NCORES = 8
BPC = B // NCORES  # rows per core
PPB = 128  # SBUF partitions
ROWS_PP = PPB // BPC  # partitions per row (8)
FREE = BPC * N * C // PPB  # bytes per partition (98304)
LN_EPS = 1e-5

# DMA jobs: bytes per partition. Small ramp-in so compute starts early,
# small ramp-out so the post-final-DMA PE tail is one q-slice.
JOB_SIZES = [1536, 3072, 4608] + [9216] * 9 + [4608, 1536]
# tokens the Vector engine takes from the tail of each job's token range
DVE_SHARE = {2: 4, 3: 6, 4: 6, 5: 6, 6: 6, 7: 6, 8: 4, 9: 2}

_CACHE = {}


def _build():
    import concourse.bass as bass  # noqa: F401
    from concourse import bacc, mybir

    f32 = mybir.dt.float32
    f32r = mybir.dt.float32r
    f8 = mybir.dt.float8e4
    DR = mybir.MatmulPerfMode.DoubleRow
    AX = mybir.AxisListType

    assert sum(JOB_SIZES) == FREE

    nc = bacc.Bacc(
        "TRN2", target_bir_lowering=False, debug=False, num_devices=NCORES
    )
    tf = nc.dram_tensor("tf", [PPB, FREE], f8, kind="ExternalInput")
    negnr = nc.dram_tensor("negnr", [1, C], f32r, kind="ExternalInput")
    ones16 = nc.dram_tensor("ones16", [1, BPC], f32r, kind="ExternalInput")
    emat = nc.dram_tensor("emat", [PPB, 2 * BPC], f8, kind="ExternalInput")
    ematf = nc.dram_tensor("ematf", [PPB, BPC], f32r, kind="ExternalInput")
    out = nc.dram_tensor("out", [BPC, 1], f32, kind="ExternalOutput")

    # job table: (lo_byte, pe_lo, pe_hi, dve_lo, dve_hi) per-partition
    jobs = []
    lo = 0
    for k, sz in enumerate(JOB_SIZES):
        tok = sz // C
        dve = DVE_SHARE.get(k, 0)
        pe = tok - dve
        assert pe % 2 == 0 and pe >= 0
        jobs.append((lo, pe, dve))
        lo += sz
    n_dve_groups = sum(1 for k in DVE_SHARE)

    with nc.cleanup_on_exit():
        tsb = nc.alloc_sbuf_tensor("tsb", [PPB, FREE], f8)
        emat_sb = nc.alloc_sbuf_tensor("emat_sb", [PPB, 2 * BPC], f8)
        ematf_sb = nc.alloc_sbuf_tensor("ematf_sb", [PPB, BPC], f32r)
        negnr_sb = nc.alloc_sbuf_tensor("negnr_sb", [1, C], f32r)
        ones16_sb = nc.alloc_sbuf_tensor("ones16_sb", [1, BPC], f32r)
        # f32r so the DVE partials feed the f32r fold matmul directly (the
        # BIR verifier requires f32r-rounded producers); the ~2^-11 rounding
        # on partial sums is far below the fp8 input noise
        acc_a = nc.alloc_sbuf_tensor("acc_a", [PPB, C], f32r)
        acc_b = nc.alloc_sbuf_tensor("acc_b", [PPB, C], f32r)
        dtmps = [
            nc.alloc_sbuf_tensor(f"dtmp{i}", [PPB, C], f32r) for i in range(3)
        ]
        sq = nc.alloc_sbuf_tensor("sq", [BPC, C], f32)
        s = nc.alloc_sbuf_tensor("s", [BPC, 1], f32)
        ps = nc.alloc_psum_tensor("ps", [BPC, C], f32)

        esem = nc.alloc_semaphore("esem")  # emat (first sync-ring job)
        dsem = [nc.alloc_semaphore(f"dsem{k}") for k in range(len(jobs))]
        csem = nc.alloc_semaphore("csem")  # scalar-ring consts
        vsem = nc.alloc_semaphore("vsem")  # DVE instruction completions
        msem = nc.alloc_semaphore("msem")  # all matmuls done
        asem = nc.alloc_semaphore("asem")  # epilogue ACT done
        osem = nc.alloc_semaphore("osem")  # output DMA done

        with nc.Block() as blk:

            @blk.sync
            def _(eng):
                # emat rides first on the bulk ring: tiny, and the first
                # matmul needs it
                eng.dma_start(emat_sb[:], emat.ap()).then_inc(esem, 16)
                for k, (lo_, pe, dve) in enumerate(jobs):
                    sz = JOB_SIZES[k]
                    eng.dma_start(
                        tsb[:, lo_ : lo_ + sz], tf.ap()[:, lo_ : lo_ + sz]
                    ).then_inc(dsem[k], 16)

            @blk.scalar
            def _(eng):
                eng.dma_start(ematf_sb[:], ematf.ap()).then_inc(csem, 16)
                eng.dma_start(negnr_sb[:], negnr.ap()).then_inc(csem, 16)
                eng.dma_start(ones16_sb[:], ones16.ap()).then_inc(csem, 16)
                # epilogue: square + row-sum of PSUM once every matmul landed
                eng.wait_ge(msem, 1)
                eng.activation(
                    out=sq[:],
                    in_=ps[:],
                    func=mybir.ActivationFunctionType.Square,
                    scale=1.0 / N,
                    accum_out=s[:],
                ).then_inc(asem, 1)
                eng.wait_ge(asem, 1)
                eng.dma_start(out.ap(), s[:]).then_inc(osem, 16)

            @blk.vector
            def _(eng):
                # Software-pipelined reduce/add interleave: reduces cycle a
                # ring of 3 temp tiles, each add waits (via vsem counting) on
                # exactly the two producers it reads; the intervening reduce
                # covers the engine's pipelined-writeback latency so no wait
                # actually stalls.
                dve_jobs = [k for k, (_, _, dve) in enumerate(jobs) if dve]
                n_red = len(dve_jobs)
                idx = 0  # instructions issued so far (== vsem when all done)
                red_idx = {}  # reduce number -> vsem count at completion
                add_idx = {}
                n_add_issued = 0

                def issue_reduce(rn):
                    nonlocal idx
                    k = dve_jobs[rn]
                    lo_, pe, dve = jobs[k]
                    eng.wait_ge(dsem[k], 16)
                    off = lo_ + pe * C
                    g = tsb[:, off : off + dve * C].rearrange(
                        "p (t c) -> p c t", c=C
                    )
                    dst = acc_a if rn == 0 else dtmps[(rn - 1) % 3]
                    with nc.allow_low_precision(reason="fp8 stream partials"):
                        ins = eng.reduce_sum(dst[:], g, axis=AX.X)
                    ins.then_inc(vsem, 1)
                    idx += 1
                    red_idx[rn] = idx

                def issue_add(an):
                    # add an (1-based): acc = acc_prev + dtmp of reduce an
                    nonlocal idx
                    prev = acc_a if an == 1 else (acc_b, acc_a)[an % 2]
                    dst = (acc_a, acc_b)[an % 2]
                    dep = red_idx[an] if an == 1 else max(
                        red_idx[an], add_idx[an - 1]
                    )
                    eng.wait_ge(vsem, dep)
                    eng.tensor_add(
                        dst[:], prev[:], dtmps[(an - 1) % 3][:]
                    ).then_inc(vsem, 1)
                    idx += 1
                    add_idx[an] = idx

                for rn in range(n_red):
                    issue_reduce(rn)
                    # keep two reduces in flight ahead of each add
                    if rn >= 2:
                        issue_add(rn - 1)
                        n_add_issued = rn - 1
                for an in range(n_add_issued + 1, n_red):
                    issue_add(an)
                _CACHE["dve_final"] = (acc_a, acc_b)[(n_red - 1) % 2].name
                _CACHE["dve_total"] = idx

            @blk.tensor
            def _(eng):
                eng.wait_ge(esem, 16)
                lhsT = emat_sb[:].rearrange("p (j m) -> p j m", j=2)
                started = False
                for k, (lo_, pe, dve) in enumerate(jobs):
                    eng.wait_ge(dsem[k], 16)
                    for q in range(pe // 2):
                        pair = tsb[
                            :, lo_ + q * 2 * C : lo_ + (q + 1) * 2 * C
                        ].rearrange("p (j c) -> p j c", j=2)
                        eng.matmul(
                            ps[:, 0:512],
                            lhsT,
                            pair[:, :, 0:512],
                            start=not started,
                            stop=False,
                            perf_mode=DR,
                            skip_group_check=True,
                        )
                        eng.matmul(
                            ps[:, 512:C],
                            lhsT,
                            pair[:, :, 512:C],
                            start=not started,
                            stop=False,
                            perf_mode=DR,
                            skip_group_check=True,
                        )
                        started = True
                    if k == 3:
                        # fold -N*r into the accumulation, off the tail
                        eng.wait_ge(csem, 48)
                        eng.matmul(
                            ps[:, 0:512],
                            ones16_sb[:],
                            negnr_sb[:, 0:512],
                            start=False,
                            stop=False,
                            skip_group_check=True,
                        )
                        eng.matmul(
                            ps[:, 512:C],
                            ones16_sb[:],
                            negnr_sb[:, 512:C],
                            start=False,
                            stop=False,
                            skip_group_check=True,
                        )
                # fold the DVE partial: PSUM += onehot.T @ acc
                eng.wait_ge(vsem, _CACHE["dve_total"])
                acc = acc_a if _CACHE["dve_final"] == "acc_a" else acc_b
                accr = acc[:]
                eng.matmul(
                    ps[:, 0:512],
                    ematf_sb[:],
                    accr[:, 0:512],
                    start=False,
                    stop=True,
                    skip_group_check=True,
                )
                eng.matmul(
                    ps[:, 512:C],
                    ematf_sb[:],
                    accr[:, 512:C],
                    start=False,
                    stop=True,
                    skip_group_check=True,
                ).then_inc(msem, 1)

    nc.compile()
    return nc


def _get_nc():
    nc = _CACHE.get("nc")
    if nc is None:
        nc = _build()
        _CACHE["nc"] = nc
    return nc


def _host_r(mask_token, ln_w, ln_b, W1, b1, W2, b2):
    """r = Linear2(gelu_exact(Linear1(LayerNorm(mask_token)))) — one 768-vec."""
    mt = np.asarray(mask_token, np.float64).reshape(C)
    mu = mt.mean()
    var = ((mt - mu) ** 2).mean()
    x = (mt - mu) / np.sqrt(var + LN_EPS) * np.asarray(ln_w, np.float64) + np.asarray(
        ln_b, np.float64
    )
    h = x @ np.asarray(W1, np.float64) + np.asarray(b1, np.float64)
    erf = np.frompyfunc(math.erf, 1, 1)
    g = h * 0.5 * (1.0 + erf(h / math.sqrt(2.0)).astype(np.float64))
    r = g @ np.asarray(W2, np.float64) + np.asarray(b2, np.float64)
    return r.astype(np.float32)


def kernel(
    appearance_tokens,
    target_features,
    noise,
    mask_token,
    ln_w,
    ln_b,
    W1,
    b1,
    W2,
    b2,
):
    from concourse.bass_utils import run_bass_kernel_spmd

    nc = _get_nc()

    r = _host_r(mask_token, ln_w, ln_b, W1, b1, W2, b2)
    in_maps = [
        {"tf": tfull_i, **_const_inputs(r)} for tfull_i in _shard_tf(target_features)
    ]

    res = run_bass_kernel_spmd(nc, in_maps, list(range(NCORES)))
    total = 0.0
    for i in range(NCORES):
        total += float(np.asarray(res.results[i]["out"], np.float64).sum())

    loss = 2.0 * total / C / (256.0 + 1e-8)
    return np.float32(loss)


def _const_inputs(r):
    """Constant device inputs derived from the decoder vector r."""
    import ml_dtypes

    negnr = np.ascontiguousarray(-float(N) * r.reshape(1, C), np.float32)
    ones16 = np.ones((1, BPC), np.float32)
    p = np.arange(PPB)
    # DoubleRow one-hot row-selector: w[p, j*16+m] = 1 iff m == p//8
    emat = np.zeros((PPB, 2 * BPC), np.float32)
    for j in range(2):
        emat[p, j * BPC + p // ROWS_PP] = 1.0
    # f32r one-hot for the DVE-partial fold
    ematf = np.zeros((PPB, BPC), np.float32)
    ematf[p, p // ROWS_PP] = 1.0
    return {
        "negnr": negnr,
        "ones16": ones16,
        "emat": emat.astype(ml_dtypes.float8_e4m3),
        "ematf": ematf,
    }


def _shard_tf(target_features):
    import ml_dtypes

    x8 = np.ascontiguousarray(target_features, np.float32).astype(
        ml_dtypes.float8_e4m3
    )
    return x8.reshape(NCORES, PPB, FREE)
